# revision 20
# baseline (speedup 1.0000x reference)
"""Causal multi-head attention Bass/Tile kernel for Trainium2, SPMD over 8 cores.

Problem (full shapes, hardcoded):
    x  [B=4, N=2048, D=1024] f32;  Wq [1024,1024];  Wkv [1024,2048];
    Wo [1024,1024];  bo [1024];  16 heads x 64 dim;  causal softmax.

Sharding (hint: batch + head tensor-parallel):
    8 cores = 4 batches x 2 head-groups.  Core c: batch c//2, heads
    (c%2)*8..(c%2)*8+7.  Wq/Wkv column-parallel, Wo row-parallel; the
    row-parallel partial sums + bias are reduced at unshard time on host
    (each pair of cores produces a partial [N, D] for its batch).

Per-core kernel (all layouts transposed: feature dim on partitions):
    xT [D, N] -> QT/KT [IL, N] (d-on-partition), V [N, IL] (token-on-partition,
    with a ones column appended per head for softmax denominators).
    S^T tiles [j=128, i=512] = KT_tile^T-free matmul; exp on ACT (no max
    subtraction: |S*scale| <= ~2, exp is safe); causal mask multiply on DVE;
    O'^T [65, 512] = V'^T @ P^T accumulated over j in PSUM (row 64 = softmax
    denominator); normalize via DVE reciprocal + GPSIMD partition broadcast;
    out^T = Wo^T @ A^T.  Matmul operands are float16 (full-rate PE path,
    end-to-end rel err ~4e-4); PSUM accumulation is fp32 throughout.
"""

import numpy as np

import concourse.bass as bass
import concourse.bacc as bacc
import concourse.mybir as mybir
from concourse.tile import TileContext

F32 = mybir.dt.float32
# dtype used for all matmul operands (PSUM accumulation is always fp32).
# float16: 1 cycle/row on the PE (2x the float32r rate) with 10 mantissa
# bits; end-to-end rel err ~4e-4 on HW (float32r: 2.2e-4 but ~15% slower;
# bfloat16: same speed, 3e-3). Value ranges (|x|<6, |W|<0.04, p<5) are
# safely inside fp16 normal range.
MM_DT = mybir.dt.float16

FULL_CFG = dict(
    DM=1024,   # model dim
    NTOK=2048, # tokens per core (one batch)
    HL=8,      # local heads
    DH=64,     # head dim
)


def build_nc(cfg=FULL_CFG, mm_dtype=None):
    if mm_dtype is None:
        mm_dtype = MM_DT
    DM, NTOK, HL, DH = cfg["DM"], cfg["NTOK"], cfg["HL"], cfg["DH"]
    IL = HL * DH            # local inner dim
    KO = DM // 128          # contraction k-tiles for projections
    DC = IL // 128          # feature chunks of QT/KT (and AT)
    ITILE = 512
    NTI = NTOK // ITILE     # i-tiles (query blocks)
    NTJ = NTOK // 128       # j-tiles (key blocks)
    CC = DM // 128          # output feature chunks
    VW = DH + 1             # V plus ones column
    SCALE = DH ** -0.5

    assert IL % 128 == 0 and NTOK % ITILE == 0 and DM % 128 == 0

    nc = bacc.Bacc(None, target_bir_lowering=False)
    MDT = mm_dtype

    xT_d = nc.dram_tensor("xT", [DM, NTOK], MDT, kind="ExternalInput")
    wq_d = nc.dram_tensor("wq", [DM, IL], MDT, kind="ExternalInput")
    wk_d = nc.dram_tensor("wk", [DM, IL], MDT, kind="ExternalInput")
    wv_d = nc.dram_tensor("wv", [DM, IL], MDT, kind="ExternalInput")
    wo_d = nc.dram_tensor("wo", [IL, DM], MDT, kind="ExternalInput")
    # mask2[p, m, jj*ITILE + i] = 1.0 where key (2m+jj)*128+p <= query i
    mask_d = nc.dram_tensor("mask2", [128, 2, 2 * ITILE], MDT, kind="ExternalInput")
    outT_d = nc.dram_tensor("outT", [DM, NTOK], F32, kind="ExternalOutput")

    def mm(out, lhsT, rhs, **kw):
        nc.tensor.matmul(out, lhsT, rhs, **kw)

    with TileContext(nc) as tc:
        with tc.tile_pool(name="persist", bufs=1) as persist:
            QT = persist.tile([128, DC, NTOK], MDT)   # q^T, d-on-partition
            KT = persist.tile([128, DC, NTOK], MDT)   # k^T, d-on-partition
            Vb = persist.tile([128, NTJ, HL * VW], MDT)  # v', token-on-partition

            # ---------------- projections ----------------
            with (
                tc.tile_pool(name="xpool", bufs=1) as xpool,
                tc.tile_pool(name="wpool", bufs=3) as wpool,
                tc.tile_pool(name="ppsum", bufs=4, space="PSUM") as ppsum,
            ):
                xTs = xpool.tile([128, KO, NTOK], MDT)
                # two DMAs so first-half compute can start earlier
                kh = max(1, KO // 2)
                nc.sync.dma_start(
                    xTs[:, :kh, :],
                    xT_d[: kh * 128, :].rearrange("(ko p) n -> p ko n", p=128),
                )
                if KO > kh:
                    nc.sync.dma_start(
                        xTs[:, kh:, :],
                        xT_d[kh * 128:, :].rearrange("(ko p) n -> p ko n", p=128),
                    )

                # ones columns of V' (once): memset f32 scratch, then a
                # converting copy (walrus requires fp32r consumers' producers
                # to round; MEMSET can't emit fp32r directly)
                ones_s = wpool.tile([128, NTJ], F32, tag="ones", bufs=1, name="ones_s")
                nc.vector.memset(ones_s[:], 1.0)
                for h in range(HL):
                    nc.vector.tensor_copy(
                        Vb[:, :, h * VW + DH:h * VW + DH + 1], ones_s[:, :, None]
                    )

                def load_w_half(dram, half, width):
                    # [128, KO, width] slice of a [DM, 2*width]-ish weight
                    wt = wpool.tile([128, KO, width], MDT, tag="w", name="wt")
                    nc.sync.dma_start(
                        wt[:],
                        dram[:, half * width:(half + 1) * width].rearrange(
                            "(ko p) d -> p ko d", p=128
                        ),
                    )
                    return wt

                # Q/K projections: out QT/KT [dchunk, tokens].
                # (walrus rejects N>512 moving operands: s3d3_mm_num_elements)
                PROJ_N = min(512, NTOK)
                halves = max(1, DC // 2)  # dchunks per weight-half
                for dst, wdram in ((QT, wq_d), (KT, wk_d)):
                    for half in range(DC // halves if DC > 1 else 1):
                        wt = load_w_half(wdram, half, halves * 128)
                        for dc in range(halves):
                            gdc = half * halves + dc
                            for t in range(NTOK // PROJ_N):
                                ps = ppsum.tile([128, PROJ_N], F32, tag="pp", name="ps")
                                for k in range(KO):
                                    mm(
                                        ps[:],
                                        wt[:, k, dc * 128:(dc + 1) * 128],
                                        xTs[:, k, t * PROJ_N:(t + 1) * PROJ_N],
                                        start=(k == 0),
                                        stop=(k == KO - 1),
                                    )
                                nc.vector.tensor_copy(
                                    dst[:, gdc, t * PROJ_N:(t + 1) * PROJ_N], ps[:]
                                )

                # V projection: out V [tokens, dfeat], strided into Vb slots
                wv_t = wpool.tile([128, KO, IL], MDT, tag="wv", name="wv_t", bufs=1)
                nc.sync.dma_start(
                    wv_t[:], wv_d[:, :].rearrange("(ko p) d -> p ko d", p=128)
                )
                for j in range(NTJ):
                    ps = ppsum.tile([128, IL], F32, tag="pp", name="ps")
                    for k in range(KO):
                        mm(
                            ps[:, :IL],
                            xTs[:, k, j * 128:(j + 1) * 128],
                            wv_t[:, k, :],
                            start=(k == 0),
                            stop=(k == KO - 1),
                        )
                    nc.vector.tensor_copy(
                        Vb[:, j, :].rearrange("p (h w) -> p h w", w=VW)[:, :, :DH],
                        ps[:, :IL].rearrange("p (h d) -> p h d", d=DH),
                    )

            # ---------------- attention + out-projection ----------------
            # Heads processed in pairs (h0 at partitions 0:64, h1 at 64:128):
            # the two dots matmuls land in different PE row-groups and run
            # concurrently, and keep the full array active for the HAM clock.
            with (
                tc.tile_pool(name="attn", bufs=2) as attn,
                tc.tile_pool(name="ptpool", bufs=4) as ptpool,
                tc.tile_pool(name="spsum", bufs=2, space="PSUM") as spsum,
                tc.tile_pool(name="opsum", bufs=2, space="PSUM") as opsum,
                tc.tile_pool(name="qpsum", bufs=2, space="PSUM") as qpsum,
            ):
                masks = attn.tile([128, 2, 2 * ITILE], MDT, bufs=1)
                nc.sync.dma_start(masks[:], mask_d[:, :, :])
                wo_t = attn.tile([128, DC, DM], MDT, bufs=1)
                nc.sync.dma_start(
                    wo_t[:], wo_d[:, :].rearrange("(mk p) c -> p mk c", p=128)
                )

                for t in range(NTI):
                    AT_t = attn.tile([128, DC, ITILE], MDT, tag="at", name="AT_t")
                    isl = slice(t * ITILE, (t + 1) * ITILE)
                    for hp in range(HL // 2):
                        h0, h1 = 2 * hp, 2 * hp + 1
                        hc = hp
                        osum0 = opsum.tile([128, ITILE], F32, tag="os", name="osum0")
                        osum1 = opsum.tile([128, ITILE], F32, tag="os", name="osum1")
                        npairs = (t + 1) * (ITILE // 256)  # 2t+2 when ITILE=512
                        for jp in range(npairs):
                            s2a = spsum.tile([128, 1024], F32, tag="s2", name="s2a")
                            s2b = spsum.tile([128, 1024], F32, tag="s2", name="s2b")
                            for jj in range(2):
                                j = 2 * jp + jj
                                jsl = slice(j * 128, (j + 1) * 128)
                                osl = slice(jj * 512, (jj + 1) * 512)
                                mm(s2a[:, osl], KT[0:DH, hc, jsl],
                                   QT[0:DH, hc, isl], start=True, stop=True)
                                mm(s2b[:, osl], KT[DH:2 * DH, hc, jsl],
                                   QT[DH:2 * DH, hc, isl], start=True, stop=True)
                            pta = ptpool.tile([128, 1024], MDT, tag="pt", name="pta")
                            ptb = ptpool.tile([128, 1024], MDT, tag="pt", name="ptb")
                            nc.scalar.activation(
                                pta[:], s2a[:],
                                mybir.ActivationFunctionType.Exp, scale=SCALE)
                            nc.scalar.activation(
                                ptb[:], s2b[:],
                                mybir.ActivationFunctionType.Exp, scale=SCALE)
                            if jp >= npairs - 2:
                                m = masks[:, jp - (npairs - 2), :]
                                nc.vector.tensor_mul(pta[:], pta[:], m)
                                nc.vector.tensor_mul(ptb[:], ptb[:], m)
                            for jj in range(2):
                                j = 2 * jp + jj
                                osl = slice(jj * 512, (jj + 1) * 512)
                                st = dict(start=(jp == 0 and jj == 0),
                                          stop=(jp == npairs - 1 and jj == 1))
                                mm(osum0[:VW, :], Vb[:, j, h0 * VW:(h0 + 1) * VW],
                                   pta[:, osl], **st)
                                mm(osum1[:VW, :], Vb[:, j, h1 * VW:(h1 + 1) * VW],
                                   ptb[:, osl], **st)
                        # normalize pair: A^T = O / sigma (sigma in [1, ~2e3],
                        # so the ~51-ULP fast reciprocal is more than enough).
                        # The custom-DVE reciprocal mis-addresses non-zero-base
                        # PSUM inputs (HW-verified) — stage sigma into SBUF first.
                        sg_a = ptpool.tile([1, ITILE], F32, tag="sa", name="sg_a", bufs=2)
                        sg_b = ptpool.tile([1, ITILE], F32, tag="sb", name="sg_b", bufs=2)
                        nc.vector.tensor_copy(sg_a[:], osum0[DH:DH + 1, :])
                        nc.vector.tensor_copy(sg_b[:], osum1[DH:DH + 1, :])
                        rden_a = ptpool.tile([1, ITILE], F32, tag="ra", name="rden_a", bufs=2)
                        rden_b = ptpool.tile([1, ITILE], F32, tag="rb2", name="rden_b", bufs=2)
                        nc.vector.reciprocal_approx_fast(rden_a[:], sg_a[:])
                        nc.vector.reciprocal_approx_fast(rden_b[:], sg_b[:])
                        # partition_broadcast writes garbage when the output
                        # slice starts at partition 64 (HW-verified) — use two
                        # base-0 tiles instead
                        rb_a = ptpool.tile([DH, ITILE], F32, tag="rba", name="rb_a", bufs=2)
                        rb_b = ptpool.tile([DH, ITILE], F32, tag="rbb", name="rb_b", bufs=2)
                        nc.gpsimd.partition_broadcast(rb_a[:], rden_a[0:1, :])
                        nc.gpsimd.partition_broadcast(rb_b[:], rden_b[0:1, :])
                        nc.vector.tensor_mul(
                            AT_t[0:DH, hc, :], osum0[:DH, :], rb_a[:])
                        nc.vector.tensor_mul(
                            AT_t[DH:2 * DH, hc, :], osum1[:DH, :], rb_b[:])
                    # out-projection for this i-tile
                    for c in range(CC):
                        ops = qpsum.tile([128, ITILE], F32, tag="op", name="ops")
                        for mk in range(DC):
                            mm(
                                ops[:],
                                wo_t[:, mk, c * 128:(c + 1) * 128],
                                AT_t[:, mk, :],
                                start=(mk == 0),
                                stop=(mk == DC - 1),
                            )
                        stg = ptpool.tile([128, ITILE], F32, tag="stg", name="stg")
                        nc.vector.tensor_copy(stg[:], ops[:])
                        nc.sync.dma_start(outT_d[c * 128:(c + 1) * 128, isl], stg[:])

    nc.finalize()
    return nc


def make_masks(itile=512):
    """mask2[p, m, jj*itile + i] = 1.0 iff (2m+jj)*128 + p <= i."""
    p = np.arange(128)[:, None]
    i = np.arange(itile)[None, :]
    out = np.zeros((128, 2, 2 * itile), np.float32)
    for m in range(2):
        for jj in range(2):
            a = 2 * m + jj
            out[:, m, jj * itile:(jj + 1) * itile] = (a * 128 + p <= i)
    return out


def shard_inputs(x, Wq, Wkv, Wo, np_dtype=np.float32):
    """Per-core input maps: core c -> batch c//2, head-group c%2."""
    B = x.shape[0]
    IL = Wq.shape[1] // 2
    D = Wq.shape[0]
    mask2 = make_masks().astype(np_dtype)
    in_maps = []
    for c in range(2 * B):
        b, hg = c // 2, c % 2
        in_maps.append({
            "xT": np.ascontiguousarray(x[b].T).astype(np_dtype),
            "wq": np.ascontiguousarray(Wq[:, hg * IL:(hg + 1) * IL]).astype(np_dtype),
            "wk": np.ascontiguousarray(Wkv[:, hg * IL:(hg + 1) * IL]).astype(np_dtype),
            "wv": np.ascontiguousarray(Wkv[:, D + hg * IL:D + (hg + 1) * IL]).astype(np_dtype),
            "wo": np.ascontiguousarray(Wo[hg * IL:(hg + 1) * IL, :]).astype(np_dtype),
            "mask2": mask2,
        })
    return in_maps


_CACHED = {}


def kernel(x, Wq, Wkv, Wo, bo):
    from concourse.bass_utils import run_bass_kernel_spmd

    x = np.asarray(x, np.float32)
    Wq = np.asarray(Wq, np.float32)
    Wkv = np.asarray(Wkv, np.float32)
    Wo = np.asarray(Wo, np.float32)
    bo = np.asarray(bo, np.float32)

    if "nc" not in _CACHED:
        _CACHED["nc"] = build_nc()
    nc = _CACHED["nc"]

    in_maps = shard_inputs(x, Wq, Wkv, Wo, np_dtype=mybir.dt.np(MM_DT))
    res = run_bass_kernel_spmd(nc, in_maps, core_ids=list(range(8)))

    B, N, D = x.shape
    out = np.empty((B, N, D), np.float32)
    for b in range(B):
        acc = res.results[2 * b]["outT"].astype(np.float32) + \
              res.results[2 * b + 1]["outT"].astype(np.float32)
        out[b] = acc.T + bo
    return out
